# revision 27
# baseline (speedup 1.0000x reference)
"""Trainium2 Bass kernel for local (block-sparse) scaled-dot-product attention.

Contract: kernel(**inputs) takes the FULL inputs of the reference
(query/key_in/value [8, 4096, 512] fp32, Wq/Wk/Wv/Wo [512, 512], biases [512])
and returns the FULL output [8, 4096, 512] fp32.

Sharding: data-parallel over batch; batch element b runs on NeuronCore b.

On-chip layout is feature-major ("transposed"): activations live as [feat, t]
so the contraction dim of every matmul is on partitions. The CPU pre-transposes
the inputs/weights (free) and transposes the output back.

Key structure (per core, per t-group of 512 positions):
 - q/k projections feature-major in fp8 (DoubleRowSwInterleave, 2x PE rate);
   v/o projections bf16 (fp8 there fails the accuracy budget).
 - Per block n, scores sT[j, q] over the 128-wide key window [64n-32, 64n+96).
 - Window masks fold into the exp activation as per-partition bias vectors
   applied to q-halves (no mask matmuls).
 - Softmax normalization happens *after* PV: oT_raw = sum_k e_k v_k times the
   PE-broadcast reciprocal denominators (bc2).  bv folds into bo on the CPU
   (bo' = Wo@bv + bo; softmax weights sum to 1), so v needs no bias.
 - Emission is software-pipelined: loads prefetch one group ahead, and the
   output projection of group g is interleaved into the projections of group
   g+1 so the PE never waits on the softmax tail.
"""

import math

import numpy as np
import ml_dtypes

import concourse.bass as bass
import concourse.tile as tile
from concourse import bacc, mybir
from concourse.bass_utils import run_bass_kernel_spmd

# ---- problem constants (hardcoded; must match the reference) ----
B, T, F = 8, 4096, 512
H, DK, DV = 8, 64, 64
CTX = 64          # block size (cq == ck == 64, nb == 64)
NB = T // CTX     # 64 blocks
NEG = -1e20
SCALE = 1.0 / math.sqrt(DK)

TG = 8            # t-groups per core
TT = T // TG      # 512 t positions per group
NB8 = TT // CTX   # 8 blocks per group

DT = mybir.dt.bfloat16
NP_DT = ml_dtypes.bfloat16
F32 = mybir.dt.float32
FP8 = mybir.dt.float8e4
NP_FP8 = ml_dtypes.float8_e4m3

FP8_QK = True  # fp8 DoubleRowSwInterleave for the q/k projections
FP8_V = False  # fp8 v-proj: fails the accuracy budget (rel ~4.6e-2)
FP8_O = False  # fp8 o-proj: fails the accuracy budget (rel ~4.4e-2)
NCV = 33       # v-proj t-tiles on the -32-shifted 128 grid (covers [-32, 4192))

_CACHED = None


def _build_maskbias():
    """Per-partition fp32 bias vectors for the windowed-softmax masks.

    Scores tile sT[j, q] per block: j in 0..128 indexes keys
    [64n-32, 64n+96), q in 0..64.  exp is evaluated as
    exp(SCALE*s + bias[j]) separately on q-halves:
      A: NEG for j >= 96            (mid/last blocks, q < 32)
      B: NEG for j < 32             (mid/first blocks, q >= 32)
      C: NEG for j < 32 or j >= 96  (block 0 q<32; block 63 q>=32)
    """
    j = np.arange(128)
    mb = np.zeros((3, 128), np.float32)
    mb[0] = NEG * (j >= 96)
    mb[1] = NEG * (j < 32)
    mb[2] = NEG * ((j < 32) | (j >= 96))
    return mb


def _sumsel():
    # osel [128, 3]: columns (ones, zeros, ones); lhsT pairs (0:2) and (1:3)
    # route per-hl column sums to psum partitions 0 and 1.
    s = np.zeros((128, 3), np.float32)
    s[:, 0] = 1.0
    s[:, 2] = 1.0
    return s


def _rowsel():
    # hlsel [2, 128]: hlsel[hl, 64*hl' + d] = 1 if hl' == hl (bc2 lhsT)
    r = np.zeros((2, 128), np.float32)
    r[0, 0:64] = 1.0
    r[1, 64:128] = 1.0
    return r


def _build_nc(n_iter=1):
    nc = bacc.Bacc(None, target_bir_lowering=False, debug=False)

    qk_dt = FP8 if FP8_QK else DT
    xq = nc.dram_tensor("xq", [F, T], qk_dt, kind="ExternalInput")
    xk = nc.dram_tensor("xk", [F, T], qk_dt, kind="ExternalInput")
    if FP8_V:
        # SwInterleave activation layout (see _prep_xv8): [p, j, c, 2m+t]
        xv = nc.dram_tensor("xv", [128, 2, NCV, 256], FP8, kind="ExternalInput")
    else:
        xv = nc.dram_tensor("xv", [F, T], DT, kind="ExternalInput")
    if FP8_QK:
        # SwInterleave weight layout (see _prep_w8): [p, j, oc, 2m+t]
        wq = nc.dram_tensor("wq", [128, 2, 4, 256], FP8, kind="ExternalInput")
        wk = nc.dram_tensor("wk", [128, 2, 4, 256], FP8, kind="ExternalInput")
    else:
        wq = nc.dram_tensor("wq", [F, F], DT, kind="ExternalInput")  # Wq.T
        wk = nc.dram_tensor("wk", [F, F], DT, kind="ExternalInput")  # Wk.T
    wv = nc.dram_tensor("wv", [F, F], FP8 if FP8_V else DT, kind="ExternalInput")
    if FP8_O:
        wo = nc.dram_tensor("wo", [128, 2, 4, 256], FP8, kind="ExternalInput")
    else:
        wo = nc.dram_tensor("wo", [F, F], DT, kind="ExternalInput")  # Wo.T
    bq = nc.dram_tensor("bq", [F], F32, kind="ExternalInput")
    bk = nc.dram_tensor("bk", [F], F32, kind="ExternalInput")
    bo = nc.dram_tensor("bo", [F], F32, kind="ExternalInput")  # Wo@bv + bo
    mb = nc.dram_tensor("mb", [3, 128], F32, kind="ExternalInput")
    # osel: [128, 3] cols = (ones, zeros, ones); lhsT pairs for per-hl sums
    ss = nc.dram_tensor("ss", [128, 3], DT, kind="ExternalInput")
    # hlsel: [2, 128] hlsel[hl, 64*hl'+d] = (hl == hl'); bc2 broadcast lhsT
    rsel = nc.dram_tensor("rsel", [2, 128], DT, kind="ExternalInput")
    outd = nc.dram_tensor("out", [F, T], DT, kind="ExternalOutput")

    Exp = mybir.ActivationFunctionType.Exp
    DR = mybir.MatmulPerfMode.DoubleRowSwInterleave

    with tile.TileContext(nc) as tc:
        with (
            tc.tile_pool(name="singles", bufs=1) as singles,
            tc.tile_pool(name="xin", bufs=2) as xin,
            tc.tile_pool(name="proj_out", bufs=2) as pqk,
            tc.tile_pool(name="vpool", bufs=2) as vpool,
            tc.tile_pool(name="epool", bufs=3) as epool,
            tc.tile_pool(name="ypool", bufs=2) as ypool,
            tc.tile_pool(name="opool", bufs=2) as opool,
            tc.tile_pool(name="ps_proj", bufs=2, space="PSUM") as ps_proj,
            tc.tile_pool(name="ps_s", bufs=2, space="PSUM") as ps_s,
            tc.tile_pool(name="ps_r", bufs=2, space="PSUM") as ps_r,
        ):
            # ---- static tiles (loads spread across DMA queues) ----
            if FP8_QK:
                wq_t = singles.tile([128, 2, 4, 256], FP8, tag="wq")
                wk_t = singles.tile([128, 2, 4, 256], FP8, tag="wk")
                nc.sync.dma_start(out=wq_t, in_=wq[:, :, :, :])
                nc.sync.dma_start(out=wk_t, in_=wk[:, :, :, :])
            else:
                wq_t = singles.tile([128, 4, F], DT, tag="wq")
                wk_t = singles.tile([128, 4, F], DT, tag="wk")
                nc.sync.dma_start(
                    out=wq_t, in_=wq.rearrange("(c p) o -> p c o", p=128)
                )
                nc.sync.dma_start(
                    out=wk_t, in_=wk.rearrange("(c p) o -> p c o", p=128)
                )
            wv_t = singles.tile([128, 4, F], FP8 if FP8_V else DT, tag="wv")
            nc.scalar.dma_start(out=wv_t, in_=wv.rearrange("(c p) o -> p c o", p=128))
            if FP8_O:
                wo_t = singles.tile([128, 2, 4, 256], FP8, tag="wo")
                nc.scalar.dma_start(out=wo_t, in_=wo[:, :, :, :])
            else:
                wo_t = singles.tile([128, 4, F], DT, tag="wo")
                nc.scalar.dma_start(
                    out=wo_t, in_=wo.rearrange("(c p) o -> p c o", p=128)
                )
            bq_t = singles.tile([128, 4], F32, tag="bq")
            bk_t = singles.tile([128, 4], F32, tag="bk")
            bo_t = singles.tile([128, 4], F32, tag="bo")
            for bt, bd in ((bq_t, bq), (bk_t, bk), (bo_t, bo)):
                nc.gpsimd.dma_start(out=bt, in_=bd.rearrange("(c p) -> p c", p=128))
            mb_t = singles.tile([128, 3], F32, tag="mb")
            nc.gpsimd.dma_start(out=mb_t, in_=mb.rearrange("k p -> p k"))
            osel_t = singles.tile([128, 3], DT, tag="osel")
            nc.gpsimd.dma_start(out=osel_t, in_=ss[:, :])
            hlsel_t = singles.tile([2, 128], DT, tag="hlsel")
            nc.gpsimd.dma_start(out=hlsel_t, in_=rsel[:, :])

            xq_r = xq.rearrange("(c p) t -> p c t", p=128)
            xk_r = xk.rearrange("(c p) t -> p c t", p=128)
            xv_r = None if FP8_V else xv.rearrange("(c p) t -> p c t", p=128)
            out_r = outd.rearrange("(c p) t -> p c t", p=128)

            def qk_proj(ps, w_t, x_s, oc, lo, n):
                # ps[0:128, 0:n] += (W.T chunk).T @ x  over the 4 f-chunks
                if FP8_QK:
                    for j in range(2):
                        nc.tensor.matmul(
                            ps,
                            lhsT=w_t[:, j, oc, :],
                            rhs=x_s[:, 2 * j : 2 * j + 2, lo : lo + n],
                            start=(j == 0),
                            stop=(j == 1),
                            perf_mode=DR,
                        )
                else:
                    for fc in range(4):
                        nc.tensor.matmul(
                            ps,
                            lhsT=w_t[:, fc, oc * 128 : (oc + 1) * 128],
                            rhs=x_s[:, fc, lo : lo + n],
                            start=(fc == 0),
                            stop=(fc == 3),
                        )

            def emit_loads(tg):
                t0 = tg * TT
                xq_s = xin.tile([128, 4, TT], qk_dt, tag="xq")
                nc.sync.dma_start(out=xq_s, in_=xq_r[:, :, t0 : t0 + TT])
                lo, hi = t0 - 32, t0 + TT + 32
                clo, chi = max(lo, 0), min(hi, T)
                xk_s = xin.tile([128, 4, TT + 64], qk_dt, tag="xk")
                nc.sync.dma_start(
                    out=xk_s[:, :, clo - lo : chi - lo], in_=xk_r[:, :, clo:chi]
                )
                if clo > lo:
                    nc.vector.memset(xk_s[:, :, 0 : clo - lo], 0.0)
                if chi < hi:
                    nc.vector.memset(xk_s[:, :, TT + 64 - (hi - chi) :], 0.0)
                if FP8_V:
                    # 5 t-tiles on the -32 grid: c = 4*tg .. 4*tg+4
                    xv_s = xin.tile([128, 2, 5, 256], FP8, tag="xv")
                    nc.sync.dma_start(
                        out=xv_s, in_=xv[:, :, 4 * tg : 4 * tg + 5, :]
                    )
                else:
                    # v window extends to t0+608 so tile 4*tg+4 (the next
                    # group's boundary tile) is computed in full.
                    vlo, vhi = t0 - 32, t0 + TT + 96
                    vclo, vchi = max(vlo, 0), min(vhi, T)
                    xv_s = xin.tile([128, 4, TT + 128], DT, tag="xv")
                    xv_q = nc.scalar if tg == 0 else nc.sync
                    xv_q.dma_start(
                        out=xv_s[:, :, vclo - vlo : vchi - vlo],
                        in_=xv_r[:, :, vclo:vchi],
                    )
                    if vclo > vlo:
                        nc.vector.memset(xv_s[:, :, 0 : vclo - vlo], 0.0)
                    if vchi < vhi:
                        nc.vector.memset(
                            xv_s[:, :, TT + 128 - (vhi - vchi) :], 0.0
                        )
                return xq_s, xk_s, xv_s

            def emit_group(
                tg, tiles, next_tiles, prev=None, prev_v0=None,
                pending_tail=None,
            ):
                t0 = tg * TT
                xq_s, xk_s, xv_s = tiles

                # ---- v projection (t-major) on the -32-shifted 128 grid ----
                # Tiles c = 4*tg+1 .. 4*tg+4; tile 4*tg (the [t0-32, t0+96)
                # boundary) is reused from the previous group's v0 chunk 3.
                # First so the v0s shuffle DMA overlaps the q/k projections.
                # No bias: bv is folded into bo on the CPU.
                v0 = vpool.tile([128, 4, F], DT, tag="v0", bufs=3)
                if tg == 0:
                    vb = vpool.tile([128, F], DT, tag="vb")
                    dsts = [(vb, None, 0)]
                else:
                    vb = prev_v0[:, 3, :]
                    dsts = []
                for tc in range(4):
                    dsts.append((v0, tc, 128 * (tc + 1)))
                for i, (dst, ch, off) in enumerate(dsts):
                    ps = ps_proj.tile([128, 512], F32, tag="proj")
                    for fc in range(4):
                        nc.tensor.matmul(
                            ps,
                            lhsT=xv_s[:, fc, off : off + 128],
                            rhs=wv_t[:, fc, :],
                            start=(fc == 0),
                            stop=(fc == 3),
                        )
                    outap = dst if ch is None else dst[:, ch, :]
                    if i % 2 == 1:
                        nc.scalar.copy(out=outap, in_=ps)
                    else:
                        nc.vector.tensor_copy(out=outap, in_=ps)
                # shifted copy: v0s covers [t0+32, t0+544), chunk c = rows
                # [64..128) of grid tile c plus rows [0..64) of grid tile c+1.
                # Issued on gpsimd so they don't queue behind SP input loads.
                if pending_tail is not None:
                    p_proj, p_yT, p_eT = pending_tail
                    emit_pair_tail(p_proj, p_yT, H // 2 - 1, p_eT)
                v0s = vpool.tile([128, 4, F], DT, tag="v0s")
                nc.gpsimd.dma_start(out=v0s[0:64, 0, :], in_=vb[64:128, :])
                nc.gpsimd.dma_start(out=v0s[0:64, 1:4, :], in_=v0[64:128, 0:3, :])
                nc.gpsimd.dma_start(out=v0s[64:128, :, :], in_=v0[0:64, 0:4, :])

                # ---- q/k projections (feature-major), interleaved with the
                # previous group's output projection so the PE stays fed
                # while the fp8 q/k evacuations drain.
                qT = pqk.tile([128, 4, TT], DT, tag="qT")
                kT = pqk.tile([128, 4, TT + 64], DT, tag="kT")
                if prev is not None:
                    yT_prev, t0_prev = prev
                    outsb = opool.tile([128, 4, TT], DT, tag="outsb")
                for oc in range(4):
                    ps = ps_proj.tile([128, 512], F32, tag="proj")
                    qk_proj(ps, wq_t, xq_s, oc, 0, 512)
                    nc.scalar.add(qT[:, oc, :], ps, bq_t[:, oc : oc + 1])
                    ps = ps_proj.tile([128, 512], F32, tag="proj")
                    qk_proj(ps, wk_t, xk_s, oc, 0, 512)
                    nc.vector.tensor_scalar_add(
                        kT[:, oc, 0:512], ps, bk_t[:, oc : oc + 1]
                    )
                    # k-halo psum in ps_r (idle during projections) so
                    # ps_proj keeps double-buffering q/k.
                    ps2 = ps_r.tile([128, 64], F32, tag="r")
                    qk_proj(ps2, wk_t, xk_s, oc, 512, 64)
                    nc.scalar.add(kT[:, oc, 512:576], ps2, bk_t[:, oc : oc + 1])
                    if prev is not None:
                        pso = ps_s.tile([128, 512], F32, tag="sT")
                        if FP8_O:
                            for j in range(2):
                                nc.tensor.matmul(
                                    pso,
                                    lhsT=wo_t[:, j, oc, :],
                                    rhs=yT_prev[:, 2 * j : 2 * j + 2, :],
                                    start=(j == 0),
                                    stop=(j == 1),
                                    perf_mode=DR,
                                )
                        else:
                            for fc in range(4):
                                nc.tensor.matmul(
                                    pso,
                                    lhsT=wo_t[:, fc, oc * 128 : (oc + 1) * 128],
                                    rhs=yT_prev[:, fc, :],
                                    start=(fc == 0),
                                    stop=(fc == 3),
                                )
                        nc.vector.tensor_scalar_add(
                            outsb[:, oc, :], pso, bo_t[:, oc : oc + 1]
                        )
                if prev is not None:
                    nc.sync.dma_start(
                        out=out_r[:, :, t0_prev : t0_prev + TT], in_=outsb
                    )

                # prefetch next group's inputs while attention runs
                if next_tiles is not None:
                    next_tiles.append(emit_loads(tg + 1))

                return qT, kT, v0, v0s, vb

            def emit_pair_scores(tg, proj, hp):
                qT, kT, v0, v0s, vb = proj
                oc = hp
                # Both heads' QK matmuls adjacently: disjoint 64-row
                # contraction groups pack in the PE array.
                sT = ps_s.tile([128, 2, NB8, 64], F32, tag="sT")
                for n8 in range(NB8):
                    for hl in range(2):
                        pb = hl * 64
                        nc.tensor.matmul(
                            sT[:, hl, n8, :],
                            lhsT=kT[pb : pb + 64, oc, 64 * n8 : 64 * n8 + 128],
                            rhs=qT[pb : pb + 64, oc, 64 * n8 : 64 * n8 + 64],
                            start=True,
                            stop=True,
                        )
                # exp with mask-bias on q-halves -> eT (pair tile)
                eT = epool.tile([128, 2, NB8, 64], DT, tag="eT")
                act = nc.scalar.activation
                if tg == 0:
                    # block 0 is a 'first' block: bias C on its q-half 0
                    act(out=eT[:, :, 0, 0:32], in_=sT[:, :, 0, 0:32],
                        func=Exp, scale=SCALE, bias=mb_t[:, 2:3])
                    act(out=eT[:, :, 1:NB8, 0:32], in_=sT[:, :, 1:NB8, 0:32],
                        func=Exp, scale=SCALE, bias=mb_t[:, 0:1])
                    act(out=eT[:, :, :, 32:64], in_=sT[:, :, :, 32:64],
                        func=Exp, scale=SCALE, bias=mb_t[:, 1:2])
                elif tg == TG - 1:
                    # block 63 is a 'last' block: bias C on its q-half 1
                    act(out=eT[:, :, :, 0:32], in_=sT[:, :, :, 0:32],
                        func=Exp, scale=SCALE, bias=mb_t[:, 0:1])
                    act(out=eT[:, :, 0 : NB8 - 1, 32:64],
                        in_=sT[:, :, 0 : NB8 - 1, 32:64],
                        func=Exp, scale=SCALE, bias=mb_t[:, 1:2])
                    act(out=eT[:, :, NB8 - 1, 32:64],
                        in_=sT[:, :, NB8 - 1, 32:64],
                        func=Exp, scale=SCALE, bias=mb_t[:, 2:3])
                else:
                    act(out=eT[:, :, :, 0:32], in_=sT[:, :, :, 0:32],
                        func=Exp, scale=SCALE, bias=mb_t[:, 0:1])
                    act(out=eT[:, :, :, 32:64], in_=sT[:, :, :, 32:64],
                        func=Exp, scale=SCALE, bias=mb_t[:, 1:2])
                return eT

            def emit_pair_tail(proj, yT, hp, eT):
                qT, kT, v0, v0s, vb = proj
                oc = hp
                # per-hl per-block column sums -> [2, 8, 64]: two accumulating
                # matmuls with (ones, zeros) / (zeros, ones) lhsT pairs write
                # hl0 sums to partition 0 and hl1 sums to partition 1.
                sums = ps_r.tile([2, NB8, 64], F32, tag="r")
                nc.tensor.matmul(
                    sums, lhsT=osel_t[:, 0:2], rhs=eT[:, 0, :, :],
                    start=True, stop=False,
                )
                nc.tensor.matmul(
                    sums, lhsT=osel_t[:, 1:3], rhs=eT[:, 1, :, :],
                    start=False, stop=True,
                )
                rs = epool.tile([2, NB8, 64], DT, tag="rs")
                with nc.allow_low_precision(reason="bf16 softmax denominators"):
                    nc.vector.reciprocal(out=rs, in_=sums)
                # PV on unnormalized e (the reciprocal's DVE latency hides
                # under the PV matmuls).  oT lives in ps_proj (idle during
                # attention) so the sums/bc2 ring in ps_r never gates PV.
                oT = ps_proj.tile([128, 512], F32, tag="proj")
                for hl in range(2):
                    h = 2 * hp + hl
                    pb = hl * 64
                    for n8 in range(NB8):
                        if n8 == 0:
                            lhsT = vb[:, 64 * h : 64 * h + 64]
                        elif n8 % 2 == 0:
                            lhsT = v0[:, n8 // 2 - 1, 64 * h : 64 * h + 64]
                        else:
                            lhsT = v0s[:, (n8 - 1) // 2, 64 * h : 64 * h + 64]
                        nc.tensor.matmul(
                            oT[pb : pb + 64, 64 * n8 : 64 * n8 + 64],
                            lhsT=lhsT,
                            rhs=eT[:, hl, n8, :],
                            start=True,
                            stop=True,
                            tile_position=(0, pb),
                        )
                # broadcast reciprocals to oT layout in ONE rank-2 matmul:
                # bc2[64*hl+d, (n8, q)] = rs[hl, n8, q]
                bc2 = ps_r.tile([128, NB8, 64], F32, tag="r")
                nc.tensor.matmul(
                    bc2, lhsT=hlsel_t, rhs=rs, start=True, stop=True,
                )
                # normalize after PV: yT = oT * bc2.  The DVE multiply may
                # only take one PSUM operand, so stage bc2 in SBUF (on Pool
                # to keep Act free for exp).
                bcs = epool.tile([128, NB8, 64], DT, tag="bcs")
                nc.scalar.copy(out=bcs, in_=bc2)
                with nc.allow_low_precision(reason="fp8 o-proj input"):
                    nc.vector.tensor_mul(
                        yT[:, oc, :], oT, bcs.rearrange("p a b -> p (a b)")
                    )

            def emit_attention(tg, proj):
                yT = ypool.tile([128, 4, TT], FP8 if FP8_O else DT, tag="yT")
                prev = None
                for hp in range(H // 2):
                    eT = emit_pair_scores(tg, proj, hp)
                    if prev is not None:
                        emit_pair_tail(proj, yT, hp - 1, prev)
                    prev = eT
                # the last pair's tail is deferred into the next group's
                # v-projection (emit_group) so its exp latency hides under
                # PE work; the caller emits it for the final group.
                return yT, prev

            def emit_oproj(tg, yT):
                t0 = tg * TT
                outsb = opool.tile([128, 4, TT], DT, tag="outsb")
                for oc in range(4):
                    pso = ps_s.tile([128, 512], F32, tag="sT")
                    if FP8_O:
                        for j in range(2):
                            nc.tensor.matmul(
                                pso,
                                lhsT=wo_t[:, j, oc, :],
                                rhs=yT[:, 2 * j : 2 * j + 2, :],
                                start=(j == 0),
                                stop=(j == 1),
                                perf_mode=DR,
                            )
                    else:
                        for fc in range(4):
                            nc.tensor.matmul(
                                pso,
                                lhsT=wo_t[:, fc, oc * 128 : (oc + 1) * 128],
                                rhs=yT[:, fc, :],
                                start=(fc == 0),
                                stop=(fc == 3),
                            )
                    nc.vector.tensor_scalar_add(
                        outsb[:, oc, :], pso, bo_t[:, oc : oc + 1]
                    )
                nc.sync.dma_start(out=out_r[:, :, t0 : t0 + TT], in_=outsb)

            def emit_all():
                tiles = emit_loads(0)
                nxt = []
                proj = emit_group(0, tiles, nxt)
                for tg in range(TG):
                    yT, last_eT = emit_attention(tg, proj)
                    if tg + 1 < TG:
                        tiles = nxt[0]
                        nxt = [] if tg + 2 < TG else None
                        proj = emit_group(
                            tg + 1, tiles, nxt, prev=(yT, tg * TT),
                            prev_v0=proj[2],
                            pending_tail=(proj, yT, last_eT),
                        )
                    else:
                        emit_pair_tail(proj, yT, H // 2 - 1, last_eT)
                        emit_oproj(tg, yT)

            if n_iter == 1:
                emit_all()
            else:
                with tc.For_i(0, n_iter, 1):
                    emit_all()

    nc.finalize()
    return nc


def _get_nc(n_iter=1):
    global _CACHED
    if _CACHED is None:
        _CACHED = {}
    if n_iter not in _CACHED:
        _CACHED[n_iter] = _build_nc(n_iter)
    return _CACHED[n_iter]


def _prep_w8(W):
    """fp8 DoubleRowSwInterleave weight layout for a [F, F] weight.

    w8[p, j, oc, 2m+t] = W.T[(2j+t)*128 + p, oc*128 + (127-m)]
    (per-partition columns stored as interleaved (tile0, tile1) pairs in
    reversed column order — what the PE's SwInterleave mode consumes).
    """
    WT = np.ascontiguousarray(np.asarray(W, np.float32).T).astype(NP_FP8)
    r = WT.reshape(2, 2, 128, 4, 128)  # (j, t, p, oc, m)
    r = r[:, :, :, :, ::-1]  # reverse m
    w8 = np.stack([r[:, 0], r[:, 1]], axis=-1)  # (j, p, oc, m, t)
    w8 = w8.transpose(1, 0, 2, 3, 4).reshape(128, 2, 4, 256)
    return np.ascontiguousarray(w8)


def _prep_xv8(v):
    """fp8 SwInterleave lhsT layout for one batch's value input.

    v: [T, F] t-major. Tiles of 128 t-positions on the -32-shifted grid
    (tile c covers t in [-32+128c, 96+128c)), zero-padded outside [0, T).
    xv8[p, j, c, 2m+t] = P[(2j+t)*128 + p, 128c + (127-m)] with
    P[f, 32+t] = v[t, f].
    """
    P = np.zeros((F, NCV * 128), np.float32)
    P[:, 32 : 32 + T] = np.asarray(v, np.float32).T
    r = P.reshape(2, 2, 128, NCV, 128)  # (j, t, p, c, m-rev)
    r = r[..., ::-1]  # reverse m
    w8 = np.stack([r[:, 0], r[:, 1]], axis=-1)  # (j, p, c, m, t)
    w8 = w8.transpose(1, 0, 2, 3, 4).reshape(128, 2, NCV, 256)
    return np.ascontiguousarray(w8.astype(NP_FP8))


def _prep_in_maps(query, key_in, value, Wq, bq, Wk, bk, Wv, bv, Wo, bo):
    np_qk = NP_FP8 if FP8_QK else NP_DT
    bo_prime = (
        np.asarray(Wo, np.float32) @ np.asarray(bv, np.float32)
        + np.asarray(bo, np.float32)
    )
    if FP8_QK:
        wq_prep = _prep_w8(Wq)
        wk_prep = _prep_w8(Wk)
    else:
        wq_prep = np.ascontiguousarray(Wq.T).astype(NP_DT)
        wk_prep = np.ascontiguousarray(Wk.T).astype(NP_DT)
    if FP8_V:
        wv_prep = np.ascontiguousarray(Wv.T).astype(NP_FP8)
    else:
        wv_prep = np.ascontiguousarray(Wv.T).astype(NP_DT)
    if FP8_O:
        wo_prep = _prep_w8(Wo)
    else:
        wo_prep = np.ascontiguousarray(Wo.T).astype(NP_DT)
    shared = {
        "wq": wq_prep,
        "wk": wk_prep,
        "wv": wv_prep,
        "wo": wo_prep,
        "bq": np.asarray(bq, np.float32),
        "bk": np.asarray(bk, np.float32),
        "bo": bo_prime,
        "mb": _build_maskbias(),
        "ss": _sumsel().astype(NP_DT),
        "rsel": _rowsel().astype(NP_DT),
    }
    from concurrent.futures import ThreadPoolExecutor

    def _tp(a):
        return np.ascontiguousarray(np.asarray(a, np.float32).T.astype(NP_DT))

    def _tp8(a):
        return np.ascontiguousarray(np.asarray(a, np.float32).T.astype(np_qk))

    with ThreadPoolExecutor(12) as ex:
        xqs = list(ex.map(_tp8, [query[b] for b in range(B)]))
        xks = list(ex.map(_tp8, [key_in[b] for b in range(B)]))
        if FP8_V:
            xvs = list(ex.map(_prep_xv8, [value[b] for b in range(B)]))
        else:
            xvs = list(ex.map(_tp, [value[b] for b in range(B)]))
    in_maps = []
    for b in range(B):
        in_maps.append({"xq": xqs[b], "xk": xks[b], "xv": xvs[b], **shared})
    return in_maps


def run(trace=False, **inputs):
    nc = _get_nc()
    in_maps = _prep_in_maps(**inputs)
    res = run_bass_kernel_spmd(
        nc, in_maps, core_ids=list(range(B)), trace=trace
    )
    out = np.stack(
        [
            np.asarray(res.results[b]["out"]).astype(np.float32).T
            for b in range(B)
        ]
    )
    return out, res


def kernel(**inputs):
    out, _ = run(trace=False, **inputs)
    return out



# revision 32
# speedup vs baseline: 1.3546x; 1.3546x over previous
"""Original baseline kernel (reconstructed) for A/B loop-timing comparison."""

import math

import numpy as np
import ml_dtypes

import concourse.bass as bass
import concourse.tile as tile
from concourse import bacc, mybir
from concourse.bass_utils import run_bass_kernel_spmd

B, T, F = 8, 4096, 512
H, DK, DV = 8, 64, 64
CTX = 64
NB = T // CTX
NEG = -1e20
SCALE = 1.0 / math.sqrt(DK)

TG = 8
TT = T // TG
NB8 = TT // CTX

DT = mybir.dt.bfloat16
NP_DT = ml_dtypes.bfloat16
F32 = mybir.dt.float32
FP8 = mybir.dt.float8e4
NP_FP8 = ml_dtypes.float8_e4m3

FP8_QK = True

_CACHED = None


def _build_maskbias():
    j = np.arange(128)
    mb = np.zeros((3, 128), np.float32)
    mb[0] = NEG * (j >= 96)
    mb[1] = NEG * (j < 32)
    mb[2] = NEG * ((j < 32) | (j >= 96))
    return mb


def _sumsel():
    s = np.zeros((128, 64), np.float32)
    for n8 in range(8):
        s[:, 8 * n8 + n8] = 1.0
    return s


def _rowsel():
    r = np.zeros((8, 1024), np.float32)
    for n8 in range(8):
        r[n8, 128 * n8 : 128 * n8 + 128] = 1.0
    return r


def _build_nc(n_iter=1):
    nc = bacc.Bacc(None, target_bir_lowering=False, debug=False)

    qk_dt = FP8 if FP8_QK else DT
    xq = nc.dram_tensor("xq", [F, T], qk_dt, kind="ExternalInput")
    xk = nc.dram_tensor("xk", [F, T], qk_dt, kind="ExternalInput")
    xv = nc.dram_tensor("xv", [F, T], DT, kind="ExternalInput")
    wq = nc.dram_tensor("wq", [128, 2, 4, 256], FP8, kind="ExternalInput")
    wk = nc.dram_tensor("wk", [128, 2, 4, 256], FP8, kind="ExternalInput")
    wv = nc.dram_tensor("wv", [F, F], DT, kind="ExternalInput")
    wo = nc.dram_tensor("wo", [F, F], DT, kind="ExternalInput")
    bq = nc.dram_tensor("bq", [F], F32, kind="ExternalInput")
    bk = nc.dram_tensor("bk", [F], F32, kind="ExternalInput")
    bo = nc.dram_tensor("bo", [F], F32, kind="ExternalInput")
    mb = nc.dram_tensor("mb", [3, 128], F32, kind="ExternalInput")
    ss = nc.dram_tensor("ss", [128, 64], DT, kind="ExternalInput")
    rsel = nc.dram_tensor("rsel", [8, 1024], DT, kind="ExternalInput")
    outd = nc.dram_tensor("out", [F, T], DT, kind="ExternalOutput")

    Exp = mybir.ActivationFunctionType.Exp
    DR = mybir.MatmulPerfMode.DoubleRowSwInterleave

    with tile.TileContext(nc) as tc:
        with (
            tc.tile_pool(name="singles", bufs=1) as singles,
            tc.tile_pool(name="xin", bufs=2) as xin,
            tc.tile_pool(name="proj_out", bufs=2) as pqk,
            tc.tile_pool(name="vpool", bufs=2) as vpool,
            tc.tile_pool(name="epool", bufs=3) as epool,
            tc.tile_pool(name="ypool", bufs=2) as ypool,
            tc.tile_pool(name="opool", bufs=2) as opool,
            tc.tile_pool(name="ps_proj", bufs=2, space="PSUM") as ps_proj,
            tc.tile_pool(name="ps_s", bufs=2, space="PSUM") as ps_s,
            tc.tile_pool(name="ps_r", bufs=2, space="PSUM") as ps_r,
        ):
            wq_t = singles.tile([128, 2, 4, 256], FP8, tag="wq")
            wk_t = singles.tile([128, 2, 4, 256], FP8, tag="wk")
            nc.sync.dma_start(out=wq_t, in_=wq[:, :, :, :])
            nc.sync.dma_start(out=wk_t, in_=wk[:, :, :, :])
            wv_t = singles.tile([128, 4, F], DT, tag="wv")
            wo_t = singles.tile([128, 4, F], DT, tag="wo")
            nc.scalar.dma_start(out=wv_t, in_=wv.rearrange("(c p) o -> p c o", p=128))
            nc.scalar.dma_start(out=wo_t, in_=wo.rearrange("(c p) o -> p c o", p=128))
            bq_t = singles.tile([128, 4], F32, tag="bq")
            bk_t = singles.tile([128, 4], F32, tag="bk")
            bo_t = singles.tile([128, 4], F32, tag="bo")
            for bt, bd in ((bq_t, bq), (bk_t, bk), (bo_t, bo)):
                nc.gpsimd.dma_start(out=bt, in_=bd.rearrange("(c p) -> p c", p=128))
            mb_t = singles.tile([128, 3], F32, tag="mb")
            nc.gpsimd.dma_start(out=mb_t, in_=mb.rearrange("k p -> p k"))
            ss_t = singles.tile([128, 64], DT, tag="ss")
            nc.gpsimd.dma_start(out=ss_t, in_=ss[:, :])
            rsel_t = singles.tile([8, 1024], DT, tag="rsel")
            nc.gpsimd.dma_start(out=rsel_t, in_=rsel[:, :])

            xq_r = xq.rearrange("(c p) t -> p c t", p=128)
            xk_r = xk.rearrange("(c p) t -> p c t", p=128)
            xv_r = xv.rearrange("(c p) t -> p c t", p=128)
            out_r = outd.rearrange("(c p) t -> p c t", p=128)

            def qk_proj(ps, w_t, x_s, oc, lo, n):
                for j in range(2):
                    nc.tensor.matmul(
                        ps,
                        lhsT=w_t[:, j, oc, :],
                        rhs=x_s[:, 2 * j : 2 * j + 2, lo : lo + n],
                        start=(j == 0),
                        stop=(j == 1),
                        perf_mode=DR,
                    )

            def emit_loads(tg):
                t0 = tg * TT
                xq_s = xin.tile([128, 4, TT], qk_dt, tag="xq")
                nc.sync.dma_start(out=xq_s, in_=xq_r[:, :, t0 : t0 + TT])
                lo, hi = t0 - 32, t0 + TT + 32
                clo, chi = max(lo, 0), min(hi, T)
                xk_s = xin.tile([128, 4, TT + 64], qk_dt, tag="xk")
                nc.sync.dma_start(
                    out=xk_s[:, :, clo - lo : chi - lo], in_=xk_r[:, :, clo:chi]
                )
                if clo > lo:
                    nc.vector.memset(xk_s[:, :, 0 : clo - lo], 0.0)
                if chi < hi:
                    nc.vector.memset(xk_s[:, :, TT + 64 - (hi - chi) :], 0.0)
                # v window extends to t0+608 so tile 4*tg+4 (the next group's
                # boundary tile) is computed in full.
                vlo, vhi = t0 - 32, t0 + TT + 96
                vclo, vchi = max(vlo, 0), min(vhi, T)
                xv_s = xin.tile([128, 4, TT + 128], DT, tag="xv")
                nc.sync.dma_start(
                    out=xv_s[:, :, vclo - vlo : vchi - vlo],
                    in_=xv_r[:, :, vclo:vchi],
                )
                if vclo > vlo:
                    nc.vector.memset(xv_s[:, :, 0 : vclo - vlo], 0.0)
                if vchi < vhi:
                    nc.vector.memset(xv_s[:, :, TT + 128 - (vhi - vchi) :], 0.0)
                return xq_s, xk_s, xv_s

            def emit_group(
                tg, tiles, next_tiles, prev=None, pending_tail=None,
                prev_v0=None,
            ):
                t0 = tg * TT
                xq_s, xk_s, xv_s = tiles

                v0 = vpool.tile([128, 4, F], DT, tag="v0", bufs=3)
                if tg == 0:
                    vb = vpool.tile([128, F], DT, tag="vb")
                    dsts = [(vb, None, 0)]
                else:
                    vb = prev_v0[:, 3, :]
                    dsts = []
                for tc in range(4):
                    dsts.append((v0, tc, 128 * (tc + 1)))
                for i, (dst, ch, off) in enumerate(dsts):
                    ps = ps_proj.tile([128, 512], F32, tag="proj")
                    for fc in range(4):
                        nc.tensor.matmul(
                            ps,
                            lhsT=xv_s[:, fc, off : off + 128],
                            rhs=wv_t[:, fc, :],
                            start=(fc == 0),
                            stop=(fc == 3),
                        )
                    outap = dst if ch is None else dst[:, ch, :]
                    if i % 2 == 1:
                        nc.scalar.copy(out=outap, in_=ps)
                    else:
                        nc.vector.tensor_copy(out=outap, in_=ps)
                if pending_tail is not None:
                    p_proj, p_yT, p_eT = pending_tail
                    emit_pair_tail(p_proj, p_yT, H // 2 - 1, p_eT)
                v0s = vpool.tile([128, 4, F], DT, tag="v0s")
                nc.gpsimd.dma_start(out=v0s[0:64, 0, :], in_=vb[64:128, :])
                nc.gpsimd.dma_start(out=v0s[0:64, 1:4, :], in_=v0[64:128, 0:3, :])
                nc.gpsimd.dma_start(out=v0s[64:128, :, :], in_=v0[0:64, 0:4, :])

                qT = pqk.tile([128, 4, TT], DT, tag="qT")
                kT = pqk.tile([128, 4, TT + 64], DT, tag="kT")
                if prev is not None:
                    yT_prev, t0_prev = prev
                    outsb = opool.tile([128, 4, TT], DT, tag="outsb")
                for oc in range(4):
                    ps = ps_proj.tile([128, 512], F32, tag="proj")
                    qk_proj(ps, wq_t, xq_s, oc, 0, 512)
                    nc.vector.tensor_scalar_add(qT[:, oc, :], ps, bq_t[:, oc : oc + 1])
                    ps = ps_proj.tile([128, 512], F32, tag="proj")
                    qk_proj(ps, wk_t, xk_s, oc, 0, 512)
                    nc.vector.tensor_scalar_add(
                        kT[:, oc, 0:512], ps, bk_t[:, oc : oc + 1]
                    )
                    ps2 = ps_r.tile([128, 64], F32, tag="r")
                    qk_proj(ps2, wk_t, xk_s, oc, 512, 64)
                    nc.scalar.add(kT[:, oc, 512:576], ps2, bk_t[:, oc : oc + 1])
                    if prev is not None:
                        pso = ps_s.tile([128, 512], F32, tag="sT")
                        for fc in range(4):
                            nc.tensor.matmul(
                                pso,
                                lhsT=wo_t[:, fc, oc * 128 : (oc + 1) * 128],
                                rhs=yT_prev[:, fc, :],
                                start=(fc == 0),
                                stop=(fc == 3),
                            )
                        nc.scalar.add(outsb[:, oc, :], pso, bo_t[:, oc : oc + 1])
                if prev is not None:
                    nc.gpsimd.dma_start(
                        out=out_r[:, :, t0_prev : t0_prev + TT], in_=outsb
                    )

                if next_tiles is not None:
                    next_tiles.append(emit_loads(tg + 1))

                return qT, kT, v0, v0s, vb

            def emit_pair_scores(tg, proj, hp):
                qT, kT, v0, v0s, vb = proj
                oc = hp
                sT = ps_s.tile([128, 2, NB8, 64], F32, tag="sT")
                for n8 in range(NB8):
                    for hl in range(2):
                        pb = hl * 64
                        nc.tensor.matmul(
                            sT[:, hl, n8, :],
                            lhsT=kT[pb : pb + 64, oc, 64 * n8 : 64 * n8 + 128],
                            rhs=qT[pb : pb + 64, oc, 64 * n8 : 64 * n8 + 64],
                            start=True,
                            stop=True,
                        )
                eT = epool.tile([128, 2, NB8, 64], DT, tag="eT")
                act = nc.scalar.activation
                if tg == 0:
                    act(out=eT[:, :, 0, 0:32], in_=sT[:, :, 0, 0:32],
                        func=Exp, scale=SCALE, bias=mb_t[:, 2:3])
                    act(out=eT[:, :, 1:NB8, 0:32], in_=sT[:, :, 1:NB8, 0:32],
                        func=Exp, scale=SCALE, bias=mb_t[:, 0:1])
                    act(out=eT[:, :, :, 32:64], in_=sT[:, :, :, 32:64],
                        func=Exp, scale=SCALE, bias=mb_t[:, 1:2])
                elif tg == TG - 1:
                    act(out=eT[:, :, :, 0:32], in_=sT[:, :, :, 0:32],
                        func=Exp, scale=SCALE, bias=mb_t[:, 0:1])
                    act(out=eT[:, :, 0 : NB8 - 1, 32:64],
                        in_=sT[:, :, 0 : NB8 - 1, 32:64],
                        func=Exp, scale=SCALE, bias=mb_t[:, 1:2])
                    act(out=eT[:, :, NB8 - 1, 32:64],
                        in_=sT[:, :, NB8 - 1, 32:64],
                        func=Exp, scale=SCALE, bias=mb_t[:, 2:3])
                else:
                    act(out=eT[:, :, :, 0:32], in_=sT[:, :, :, 0:32],
                        func=Exp, scale=SCALE, bias=mb_t[:, 0:1])
                    act(out=eT[:, :, :, 32:64], in_=sT[:, :, :, 32:64],
                        func=Exp, scale=SCALE, bias=mb_t[:, 1:2])
                return eT

            def emit_pair_tail(proj, yT, hp, eT):
                qT, kT, v0, v0s, vb = proj
                oc = hp
                sums = ps_r.tile([8, 2, 64], F32, tag="r")
                for n8 in range(NB8):
                    nc.tensor.matmul(
                        sums,
                        lhsT=ss_t[:, 8 * n8 : 8 * n8 + 8],
                        rhs=eT[:, :, n8, :],
                        start=(n8 == 0),
                        stop=(n8 == NB8 - 1),
                    )
                rs = epool.tile([8, 2, 64], DT, tag="rs")
                with nc.allow_low_precision(reason="bf16 softmax denominators"):
                    nc.vector.reciprocal(out=rs, in_=sums)
                oT = ps_r.tile([128, 512], F32, tag="r")
                for hl in range(2):
                    h = 2 * hp + hl
                    pb = hl * 64
                    for n8 in range(NB8):
                        if n8 == 0:
                            lhsT = vb[:, 64 * h : 64 * h + 64]
                        elif n8 % 2 == 0:
                            lhsT = v0[:, n8 // 2 - 1, 64 * h : 64 * h + 64]
                        else:
                            lhsT = v0s[:, (n8 - 1) // 2, 64 * h : 64 * h + 64]
                        nc.tensor.matmul(
                            oT[pb : pb + 64, 64 * n8 : 64 * n8 + 64],
                            lhsT=lhsT,
                            rhs=eT[:, hl, n8, :],
                            start=True,
                            stop=True,
                            tile_position=(0, pb),
                        )
                bc2 = ps_r.tile([128, NB8, 64], F32, tag="r")
                for n8 in range(NB8):
                    for hl in range(2):
                        nc.tensor.matmul(
                            bc2[64 * hl : 64 * hl + 64, n8, :],
                            lhsT=rsel_t[:, 128 * n8 : 128 * n8 + 64],
                            rhs=rs[:, hl, :],
                            start=True,
                            stop=True,
                            tile_position=(0, 64 * hl),
                        )
                bcs = epool.tile([128, NB8, 64], DT, tag="bcs")
                nc.scalar.copy(out=bcs, in_=bc2)
                nc.vector.tensor_mul(
                    yT[:, oc, :], oT, bcs.rearrange("p a b -> p (a b)")
                )

            def emit_attention(tg, proj):
                yT = ypool.tile([128, 4, TT], DT, tag="yT")
                prev = None
                for hp in range(H // 2):
                    eT = emit_pair_scores(tg, proj, hp)
                    if prev is not None:
                        emit_pair_tail(proj, yT, hp - 1, prev)
                    prev = eT
                # last tail deferred into the next group's v-proj (or emitted
                # by the caller for the final group) so its exp latency hides
                # under PE work.
                return yT, prev

            def emit_oproj(tg, yT):
                t0 = tg * TT
                outsb = opool.tile([128, 4, TT], DT, tag="outsb")
                for oc in range(4):
                    pso = ps_s.tile([128, 512], F32, tag="sT")
                    for fc in range(4):
                        nc.tensor.matmul(
                            pso,
                            lhsT=wo_t[:, fc, oc * 128 : (oc + 1) * 128],
                            rhs=yT[:, fc, :],
                            start=(fc == 0),
                            stop=(fc == 3),
                        )
                    nc.scalar.add(outsb[:, oc, :], pso, bo_t[:, oc : oc + 1])
                nc.gpsimd.dma_start(out=out_r[:, :, t0 : t0 + TT], in_=outsb)

            def emit_all():
                tiles = emit_loads(0)
                nxt = []
                proj = emit_group(0, tiles, nxt)
                for tg in range(TG):
                    yT, last_eT = emit_attention(tg, proj)
                    if tg + 1 < TG:
                        tiles = nxt[0]
                        nxt = [] if tg + 2 < TG else None
                        proj = emit_group(
                            tg + 1, tiles, nxt, prev=(yT, tg * TT),
                            pending_tail=(proj, yT, last_eT),
                            prev_v0=proj[2],
                        )
                    else:
                        emit_pair_tail(proj, yT, H // 2 - 1, last_eT)
                        emit_oproj(tg, yT)

            if n_iter == 1:
                emit_all()
            else:
                with tc.For_i(0, n_iter, 1):
                    emit_all()

    nc.finalize()
    return nc


def _get_nc(n_iter=1):
    global _CACHED
    if _CACHED is None:
        _CACHED = {}
    if n_iter not in _CACHED:
        _CACHED[n_iter] = _build_nc(n_iter)
    return _CACHED[n_iter]


def _prep_w8(W):
    WT = np.ascontiguousarray(np.asarray(W, np.float32).T).astype(NP_FP8)
    r = WT.reshape(2, 2, 128, 4, 128)
    r = r[:, :, :, :, ::-1]
    w8 = np.stack([r[:, 0], r[:, 1]], axis=-1)
    w8 = w8.transpose(1, 0, 2, 3, 4).reshape(128, 2, 4, 256)
    return np.ascontiguousarray(w8)


def _prep_in_maps(query, key_in, value, Wq, bq, Wk, bk, Wv, bv, Wo, bo):
    bo_prime = (
        np.asarray(Wo, np.float32) @ np.asarray(bv, np.float32)
        + np.asarray(bo, np.float32)
    )
    shared = {
        "wq": _prep_w8(Wq),
        "wk": _prep_w8(Wk),
        "wv": np.ascontiguousarray(Wv.T).astype(NP_DT),
        "wo": np.ascontiguousarray(Wo.T).astype(NP_DT),
        "bq": np.asarray(bq, np.float32),
        "bk": np.asarray(bk, np.float32),
        "bo": bo_prime,
        "mb": _build_maskbias(),
        "ss": _sumsel().astype(NP_DT),
        "rsel": _rowsel().astype(NP_DT),
    }
    from concurrent.futures import ThreadPoolExecutor

    def _tp(a):
        return np.ascontiguousarray(np.asarray(a, np.float32).T.astype(NP_DT))

    def _tp8(a):
        return np.ascontiguousarray(np.asarray(a, np.float32).T.astype(NP_FP8))

    with ThreadPoolExecutor(12) as ex:
        xqs = list(ex.map(_tp8, [query[b] for b in range(B)]))
        xks = list(ex.map(_tp8, [key_in[b] for b in range(B)]))
        xvs = list(ex.map(_tp, [value[b] for b in range(B)]))
    in_maps = []
    for b in range(B):
        in_maps.append({"xq": xqs[b], "xk": xks[b], "xv": xvs[b], **shared})
    return in_maps


def run(trace=False, **inputs):
    nc = _get_nc()
    in_maps = _prep_in_maps(**inputs)
    res = run_bass_kernel_spmd(
        nc, in_maps, core_ids=list(range(B)), trace=trace
    )
    out = np.stack(
        [
            np.asarray(res.results[b]["out"]).astype(np.float32).T
            for b in range(B)
        ]
    )
    return out, res


def kernel(**inputs):
    out, _ = run(trace=False, **inputs)
    return out
